# revision 35
# baseline (speedup 1.0000x reference)
"""Multi-head attention with Llama RoPE on 8 TRN2 NeuronCores.

Problem: x [2, 2048, 1024] f32; w_qkv [3072, 1024]; w_out [1024, 1024];
b_out [1024].  16 heads x head_dim 64, full (non-causal) softmax attention.

Sharding: 8 cores = 2 batches x 4 head-groups (4 heads per core).
Each core computes q/k/v projections for its 4 heads, RoPE, attention,
and a partial output projection over its 256 local features.  The host
sums the 4 partials per batch (row-parallel out-projection); the bias is
injected on one core per batch group.

v3 structure: engine-balanced, filler-scheduled pipeline.
 - Act engine does ONLY exp (128 x [128,1024] tiles).  psum->sbuf copies
   run on gpsimd (Pool); softmax-denominator broadcast via gpsimd
   partition_broadcast (no PE ones-matmul, no Act copy).
 - Attention: qi-chunk (1024) outer, kj-tile inner, PV lagging scores by
   one tile; Act saturates while PE runs ahead on scores.
 - v is projected TRANSPOSED (wv stationary, few LDWEIGHTS) during the
   input-DMA window, then flipped to natural layout with cheap PE
   transpose ops emitted as attention fillers.
 - All remaining projection work (q/k pair 1 + rope, out-projection
   halves T1/T2) is emitted as fillers inside the attention stream; T2
   for the first s-half runs during h3 so the tail is short.
 - x / weights / rope tables in bf16 (half DMA, same PE rate).
"""
import sys

sys.path.insert(0, "/opt/trn_rl_repo")

from collections import deque
from contextlib import ExitStack

import numpy as np

import concourse.bass as bass
import concourse.tile as tile
from concourse import bacc, mybir
from concourse.bass2jax import (_bass_exec_p, install_neuronx_cc_hook,
                                partition_id_tensor)

F32 = mybir.dt.float32
F32R = mybir.dt.float32r
BF16 = mybir.dt.bfloat16

B, S, D = 2, 2048, 1024
H, HD = 16, 64          # global heads, head dim
HL = 4                  # heads per core
EL = HL * HD            # 256 local e-dims for q, k, v each
N_CORES = 8
SCP = 512               # projection / rope / vT s-chunk
N_CP = S // SCP         # 4
SCA = 1024              # attention qi chunk
N_CA = S // SCA         # 2
N_DT = D // 128         # 8 d-chunks
N_ST = S // 128         # 16 s-tiles
N_KT = S // 128         # 16 kj-tiles


def r(ap):
    return ap.bitcast(F32R)


def build_kernel(repeat=1):
    nc = bacc.Bacc(None, target_bir_lowering=False)

    # host-packed DRAM params: partition-dim 128, d-tiles packed along the
    # free axis so each load is ONE descriptor-cheap DMA
    xT_ext = nc.declare_dram_parameter("xTp", [128, N_CP * N_DT * SCP], BF16,
                                       isOutput=False)   # [c][dt][512]
    wqk_ext = nc.declare_dram_parameter("wqkp", [128, N_DT * 2 * EL], BF16,
                                        isOutput=False)  # [dt][512]
    wv_ext = nc.declare_dram_parameter("wvp", [128, N_DT * EL], BF16,
                                       isOutput=False)   # [dt][256]
    cos2_ext = nc.declare_dram_parameter("cos2", [128, S], BF16, isOutput=False)
    sin2_ext = nc.declare_dram_parameter("sin2", [128, S], BF16, isOutput=False)
    psw_ext = nc.declare_dram_parameter("psw", [128, 128], F32R, isOutput=False)
    id128_ext = nc.declare_dram_parameter("id128", [128, 128], BF16,
                                          isOutput=False)
    wo_ext = nc.declare_dram_parameter("wop", [128, 2 * D], BF16,
                                       isOutput=False)   # [eh][1024]
    bias_ext = nc.declare_dram_parameter("bias", [128, D], F32, isOutput=False)
    ones64_ext = nc.declare_dram_parameter("ones64p", [1, 64], F32R,
                                           isOutput=False)
    onesv_ext = nc.declare_dram_parameter("onesv", [128, HL * 65], BF16,
                                          isOutput=False)
    out_ext = nc.declare_dram_parameter("out", [S, D], F32, isOutput=True)

    inv_sqrt_hd = 1.0 / np.sqrt(HD)

    with tile.TileContext(nc) as tc, ExitStack() as ctx, \
            nc.allow_low_precision(reason="bf16/f32r rounding writes"):
        # ---- persistent SBUF ----
        singles = ctx.enter_context(tc.tile_pool(name="singles", bufs=1))
        xTp = singles.tile([128, N_CP * N_DT * SCP], BF16, name="xTp")
        # xT[dt] views: free dims [chunk, 512] (stride N_DT*SCP between
        # chunks); matmul/rope operands slice these to chunk granularity
        xT4 = xTp[:].rearrange("p (c d s) -> p d c s", c=N_CP, d=N_DT)
        wqkp = singles.tile([128, N_DT * 2 * EL], BF16, name="wqkp")
        wqk = [wqkp[:, 2 * EL * i:2 * EL * (i + 1)] for i in range(N_DT)]
        wvp = singles.tile([128, N_DT * EL], BF16, name="wvp")
        wv = [wvp[:, EL * i:EL * (i + 1)] for i in range(N_DT)]
        cos2 = singles.tile([128, S], BF16, name="cos2")
        sin2 = singles.tile([128, S], BF16, name="sin2")
        psw = singles.tile([128, 128], F32R, name="psw")
        id128 = singles.tile([128, 128], BF16, name="id128")
        wop = singles.tile([128, 2 * D], BF16, name="wop")
        wo = [wop[:, D * i:D * (i + 1)] for i in range(2)]
        bias = singles.tile([128, D], F32, name="bias")
        # q/k transposed [e_local, s]; index = head pair
        qT = [singles.tile([128, S], F32, name=f"qT{i}") for i in range(2)]
        kT = [singles.tile([128, S], F32, name=f"kT{i}") for i in range(2)]
        # v transposed [e_local, s] (intermediate) and natural per s-tile
        vT = [singles.tile([128, S], BF16, name=f"vT{i}") for i in range(2)]
        vsb = [singles.tile([128, HL * 65], BF16, name=f"v{i}")
               for i in range(N_ST)]
        # normalized attention output, transposed [d_local, s]
        onrm = [singles.tile([128, S], BF16, name=f"onrm{i}")
                for i in range(2)]
        # T1 = pair-0 half of the output projection (+bias), per s-tile
        t1sb = [singles.tile([128, D], BF16, name=f"t1sb{i}")
                for i in range(N_ST)]
        ones64 = singles.tile([1, 64], F32R, name="ones64")
        warm = singles.tile([1, 2], F32, name="warm")

        # ---- DMA loads: few big DMAs (DGE descriptor gen is ~650ns each),
        # ordered so the preamble's dependencies land first
        nc.sync.dma_start(out=wvp[:], in_=wv_ext[:])
        nc.sync.dma_start(out=wqkp[:], in_=wqk_ext[:])
        nc.sync.dma_start(out=psw[:], in_=psw_ext[:])
        nc.sync.dma_start(out=id128[:], in_=id128_ext[:])
        CW = N_DT * SCP
        for c in range(N_CP):
            nc.sync.dma_start(out=xTp[:, CW * c:CW * (c + 1)],
                              in_=xT_ext[:, CW * c:CW * (c + 1)])
            if c == 1:
                nc.sync.dma_start(out=cos2[:, 0:SCA], in_=cos2_ext[:, 0:SCA])
                nc.sync.dma_start(out=sin2[:, 0:SCA], in_=sin2_ext[:, 0:SCA])
        nc.sync.dma_start(out=cos2[:, SCA:S], in_=cos2_ext[:, SCA:S])
        nc.sync.dma_start(out=sin2[:, SCA:S], in_=sin2_ext[:, SCA:S])
        nc.sync.dma_start(out=wop[:], in_=wo_ext[:])
        nc.sync.dma_start(out=bias[:], in_=bias_ext[:])
        nc.sync.dma_start(out=ones64[:], in_=ones64_ext[:])

        # ---- pools ----
        # PSUM budget (8 banks): ss 2x[128,1024] = 4, po 1x[65,1024] = 2,
        # misc 2x[128,512] = 2.
        ss_pool = ctx.enter_context(tc.tile_pool(name="ss", bufs=2, space="PSUM"))
        po_pool = ctx.enter_context(tc.tile_pool(name="po", bufs=1, space="PSUM"))
        mi_pool = ctx.enter_context(tc.tile_pool(name="mi", bufs=2, space="PSUM"))
        att_sb = ctx.enter_context(tc.tile_pool(name="att_sb", bufs=10))
        rope_sb = ctx.enter_context(tc.tile_pool(name="rope_sb", bufs=2))
        nrm_sb = ctx.enter_context(tc.tile_pool(name="nrm_sb", bufs=2))
        ob_sb = ctx.enter_context(tc.tile_pool(name="ob_sb", bufs=3))

        # ones columns of v tiles (static for the whole exec)
        for st in range(N_ST):
            nc.sync.dma_start(out=vsb[st][:], in_=onesv_ext[:])
        # warm up the Act Exp table off the critical path
        nc.vector.memset(warm[:], 0.0)
        nc.scalar.activation(out=warm[:], in_=warm[:],
                             func=mybir.ActivationFunctionType.Exp)

        # ---------- emission helpers ----------
        # generator helpers yield every ~0.5-1us of PE work so filler
        # stepping never opens a long gap in the exp stream
        def emit_vt_chunk(c):
            """vT[e 256, s-chunk c] with wv stationary (2 e-halves)."""
            sl = slice(SCP * c, SCP * (c + 1))
            for eh in range(2):
                mi = mi_pool.tile([128, SCP], F32, name="vtp", tag="mi")
                for dt in range(N_DT):
                    nc.tensor.matmul(
                        mi[:],
                        wv[dt][:, 128 * eh:128 * (eh + 1)],
                        xT4[:, dt, c, :],
                        start=(dt == 0), stop=(dt == N_DT - 1),
                    )
                    if dt % 4 == 3:
                        yield
                nc.scalar.copy(out=vT[eh][:, sl], in_=mi[:])

        def emit_vtrans(st):
            """vsb[st] natural [s, h*65] from vT via PE transpose."""
            ssl = slice(128 * st, 128 * (st + 1))
            mi = mi_pool.tile([128, SCP], F32, name="vtr", tag="mi")
            tp = mi[:, 0:128].bitcast(BF16)             # [128, 256] bf16 view
            for eh in range(2):
                nc.tensor.matmul(tp[:, 128 * eh:128 * (eh + 1)],
                                 vT[eh][:, ssl], id128[:],
                                 start=True, stop=True, is_transpose=True)
            nc.vector.tensor_copy(
                out=vsb[st][:].rearrange("p (h e) -> p h e", h=HL)[:, :, 0:64],
                in_=tp.rearrange("p (h e) -> p h e", h=HL))

        def gen_qk_tile(t):
            """Project + rope q/k tile t, one 512-col s-chunk at a time.
            Uses short-lived mi-pool psum so it can interleave with the
            attention stream without touching the scores pool."""
            buf = qT[t] if t < 2 else kT[t - 2]
            for c in range(N_CP):
                sl = slice(SCP * c, SCP * (c + 1))
                mi = mi_pool.tile([128, SCP], F32, name="qkproj", tag="mi")
                for dt in range(N_DT):
                    nc.tensor.matmul(
                        mi[:],
                        wqk[dt][:, 128 * t:128 * (t + 1)],
                        xT4[:, dt, c, :],
                        start=(dt == 0), stop=(dt == N_DT - 1),
                    )
                    if dt % 4 == 3:
                        yield
                nc.scalar.copy(out=r(buf[:, sl]), in_=mi[:])
                emit_rope_chunk(t, c)
                yield

        def emit_rope_chunk(t, c):
            """Apply rope in place on q/k buf for s-chunk c (512 cols)."""
            buf = qT[t] if t < 2 else kT[t - 2]
            sl = slice(SCP * c, SCP * (c + 1))
            sw = mi_pool.tile([128, SCP], F32, name="sw", tag="mi")
            nc.tensor.matmul(sw[:], psw[:], r(buf[:, sl]), start=True, stop=True)
            tr = rope_sb.tile([128, SCP], F32, name="ropet1")
            nc.vector.tensor_mul(tr[:], buf[:, sl], cos2[:, sl])
            nc.vector.tensor_mul(r(buf[:, sl]), sw[:], sin2[:, sl])
            nc.vector.tensor_add(r(buf[:, sl]), buf[:, sl], tr[:])

        def emit_t1(st):
            """Pair-0 half of out-projection for s-tile st, +bias, to sbuf."""
            ssl = slice(128 * st, 128 * (st + 1))
            for ec in range(2):
                esl = slice(512 * ec, 512 * (ec + 1))
                ps = mi_pool.tile([128, 512], F32, name="t1ps", tag="mi")
                nc.tensor.matmul(ps[:], onrm[0][:, ssl], wo[0][:, esl],
                                 start=True, stop=True)
                nc.vector.tensor_add(t1sb[st][:, esl], ps[:], bias[:, esl])

        def emit_t2(st):
            """Pair-1 half + T1 + store."""
            ssl = slice(128 * st, 128 * (st + 1))
            ob = ob_sb.tile([128, D], F32, name="ob")
            for ec in range(2):
                esl = slice(512 * ec, 512 * (ec + 1))
                ps = mi_pool.tile([128, 512], F32, name="t2ps", tag="mi")
                nc.tensor.matmul(ps[:], onrm[1][:, ssl], wo[1][:, esl],
                                 start=True, stop=True)
                nc.vector.tensor_add(ob[:, esl], ps[:], t1sb[st][:, esl])
            nc.sync.dma_start(out=out_ext[ssl, :], in_=ob[:])

        def run(gen):
            for _ in gen:
                pass

        # ---------- preamble ----------
        # PE p-state warmup: harmless transposes while DMAs stream; the
        # Tensor engine reaches full clock after ~3us of continuous work.
        for _ in range(24):
            mi = mi_pool.tile([128, SCP], F32, name="pwarm", tag="mi")
            nc.tensor.matmul(mi[:, 0:64].bitcast(BF16), id128[:], id128[:],
                             start=True, stop=True, is_transpose=True)
        # Preamble: pair-0 k and q projected + roped chunk-by-chunk,
        # interleaved so each chunk starts as soon as its xT DMA lands.
        gk, gq = gen_qk_tile(2), gen_qk_tile(0)
        for c in range(N_CP):
            for _ in range(3):
                next(gk, None)
            for _ in range(3):
                next(gq, None)
        run(gk)
        run(gq)

        # ---------- fillers ----------
        vsb_ready = [False] * N_ST        # vtrans emitted (PE program order)

        def gen_v():
            for c in range(N_CP):
                yield from emit_vt_chunk(c)
                for st in (4 * c, 4 * c + 2):
                    emit_vtrans(st)
                    emit_vtrans(st + 1)
                    vsb_ready[st] = vsb_ready[st + 1] = True
                    yield

        def gen_t1():
            for st in range(N_ST):
                emit_t1(st)
                yield

        def gen_t2(sts):
            for st in sts:
                emit_t2(st)
                yield

        def step(fillers, n=1):
            for _ in range(n):
                while fillers:
                    try:
                        next(fillers[0])
                        break
                    except StopIteration:
                        fillers.pop(0)

        def flush(fillers):
            while fillers:
                try:
                    next(fillers[0])
                except StopIteration:
                    fillers.pop(0)

        def emit_pv(h, po, kt, at):
            nc.tensor.matmul(
                po[:],
                vsb[kt][:, 65 * h:65 * h + 65],
                at[:],
                start=(kt == 0), stop=(kt == N_KT - 1),
            )

        def attention_hc(h, c, fillers, steps=2, t2_tail=False):
            tq, ro = h // 2, 64 * (h % 2)
            asl = slice(SCA * c, SCA * (c + 1))
            po = po_pool.tile([65, SCA], F32, name="pv", tag="po")
            pend = deque()
            for kt in range(N_KT):
                ksl = slice(128 * kt, 128 * (kt + 1))
                ss = ss_pool.tile([128, SCA], F32, name="scores", tag="ss")
                nc.tensor.matmul(
                    ss[:],
                    r(kT[tq][ro:ro + 64, ksl]),
                    r(qT[tq][ro:ro + 64, asl]),
                    start=True, stop=True,
                )
                at = att_sb.tile([128, SCA], BF16, name="attnT")
                nc.scalar.activation(out=at[:], in_=ss[:],
                                     func=mybir.ActivationFunctionType.Exp,
                                     scale=inv_sqrt_hd)
                pend.append((kt, at))
                step(fillers, steps)
                # PV may only be emitted once its vsb tile's producer is in
                # the PE stream (in-order engine: emitting earlier deadlocks).
                # Larger lag at the start of each (h,c): PE must not reach
                # pv(0) before the previous norm chain has released po.
                min_lag = 6 if kt < 6 else 1
                while pend and len(pend) > min_lag and vsb_ready[pend[0][0]]:
                    emit_pv(h, po, *pend.popleft())
            while pend:
                if not vsb_ready[pend[0][0]]:
                    step(fillers, 1)
                    continue
                emit_pv(h, po, *pend.popleft())
            # deferred softmax normalization (v1-style broadcast:
            # ones-matmul on PE, psum->sbuf copy on Act)
            rec = nrm_sb.tile([1, SCA], F32, name="recip", tag="rec")
            nc.vector.reciprocal(out=r(rec[:]), in_=po[64:65, :])
            for half in range(2):
                hsl = slice(512 * half, 512 * (half + 1))
                osl = slice(SCA * c + 512 * half, SCA * c + 512 * (half + 1))
                bc = mi_pool.tile([128, 512], F32, name="bc", tag="mi")
                nc.tensor.matmul(bc[0:64, :], ones64[:], r(rec[:, hsl]),
                                 start=True, stop=True)
                bs = nrm_sb.tile([64, 512], F32, name="bcast", tag="bs")
                nc.scalar.copy(out=bs[:], in_=bc[0:64, :])
                nc.vector.tensor_mul(
                    onrm[tq][ro:ro + 64, osl], po[0:64, hsl], bs[:])
                if t2_tail:
                    for st in range(8 + 4 * half, 12 + 4 * half):
                        emit_t2(st)

        # ---------- attention schedule ----------
        fillers = [gen_v(), gen_qk_tile(3), gen_qk_tile(1)]
        attention_hc(0, 0, fillers, steps=1)
        attention_hc(0, 1, fillers, steps=1)
        attention_hc(1, 0, fillers, steps=1)
        attention_hc(1, 1, fillers, steps=1)
        flush(fillers)
        fillers = [gen_t1()]
        attention_hc(2, 0, fillers, steps=1)
        attention_hc(2, 1, fillers, steps=1)
        attention_hc(3, 0, fillers, steps=1)
        flush(fillers)                    # T1 complete before T2 starts
        fillers = [gen_t2(range(8))]
        attention_hc(3, 1, fillers, steps=1, t2_tail=True)
        flush(fillers)

    nc.finalize()
    return nc


def run_spmd_per_device(nc, in_maps):
    """8 independent single-device executions of the same NEFF (the kernel
    is pure SPMD, no collectives; the axon terminal here hangs on
    multi-device shard_map, so we dispatch per-device jits asynchronously
    instead)."""
    import jax
    install_neuronx_cc_hook()
    devs = jax.devices()[:len(in_maps)]
    partition_name = (nc.partition_id_tensor.name
                      if nc.partition_id_tensor else None)
    in_names, out_names, out_avals, zero_outs = [], [], [], []
    for alloc in nc.m.functions[0].allocations:
        if not isinstance(alloc, mybir.MemoryLocationSet):
            continue
        name = alloc.memorylocations[0].name
        if alloc.kind == "ExternalInput":
            if name != partition_name:
                in_names.append(name)
        elif alloc.kind == "ExternalOutput":
            shape = tuple(alloc.tensor_shape)
            dtype = mybir.dt.np(alloc.dtype)
            out_names.append(name)
            out_avals.append(jax.core.ShapedArray(shape, dtype))
            zero_outs.append(np.zeros(shape, dtype))
    n_params = len(in_names)
    all_names = in_names + out_names
    if partition_name is not None:
        all_names = all_names + [partition_name]

    def _body(*args):
        operands = list(args)
        if partition_name is not None:
            operands.append(partition_id_tensor())
        outs = _bass_exec_p.bind(
            *operands,
            out_avals=tuple(out_avals),
            in_names=tuple(all_names),
            out_names=tuple(out_names),
            lowering_input_output_aliases=(),
            sim_require_finite=True,
            sim_require_nnan=True,
            nc=nc,
        )
        return tuple(outs)

    donate = tuple(range(n_params, n_params + len(out_names)))
    pending = []
    for i, in_map in enumerate(in_maps):
        f = jax.jit(_body, donate_argnums=donate, keep_unused=True,
                    device=devs[i])
        args = [np.asarray(in_map[k]) for k in in_names]
        args += [z.copy() for z in zero_outs]
        pending.append(f(*args))
    return [{name: np.asarray(outs[i]) for i, name in enumerate(out_names)}
            for outs in pending]


_ROPE_TABLES = None


def _tables():
    global _ROPE_TABLES
    if _ROPE_TABLES is None:
        bf16 = mybir.dt.np(BF16)
        inv_freq = 1.0 / (10000.0 ** (np.arange(0, HD, 2, dtype=np.float32) / HD))
        t = np.arange(S, dtype=np.float32)
        freqs = np.outer(t, inv_freq).astype(np.float32)  # [S, 32]
        cos, sin = np.cos(freqs), np.sin(freqs)
        # interleave pairs: row 2i and 2i+1 both get cos_i; sin row 2i = -s_i,
        # row 2i+1 = +s_i; tile 2 heads to fill 128 partitions
        c64 = np.repeat(cos.T, 2, axis=0)                 # [64, S]
        s64 = np.repeat(sin.T, 2, axis=0).copy()
        s64[0::2, :] *= -1.0
        cos2 = np.tile(c64, (2, 1)).astype(bf16)          # [128, S]
        sin2 = np.tile(s64, (2, 1)).astype(bf16)
        psw = np.zeros((128, 128), dtype=np.float32)
        idx = np.arange(128)
        psw[idx ^ 1, idx] = 1.0                           # out[j] = in[j^1]
        _ROPE_TABLES = (cos2, sin2, psw,
                        np.eye(128, dtype=np.float32).astype(bf16))
    return _ROPE_TABLES


_NC_CACHE = None
_last_in_maps = None


def kernel(x, w_qkv, w_out, b_out):
    global _NC_CACHE
    bf16 = mybir.dt.np(BF16)
    x = np.ascontiguousarray(x, dtype=np.float32)
    w_qkv = np.asarray(w_qkv, dtype=np.float32)
    w_out = np.asarray(w_out, dtype=np.float32)
    b_out = np.asarray(b_out, dtype=np.float32)

    cos2, sin2, psw, id128 = _tables()
    wq_g = w_qkv[0 * D:1 * D].reshape(H, HD, D)
    wk_g = w_qkv[1 * D:2 * D].reshape(H, HD, D)
    wv_g = w_qkv[2 * D:3 * D].reshape(H, HD, D)

    in_maps = []
    for c in range(N_CORES):
        b, g = divmod(c, 4)
        hs = slice(4 * g, 4 * g + 4)
        wq = wq_g[hs].reshape(EL, D)                       # [256, 1024]
        wk = wk_g[hs].reshape(EL, D)
        wv = wv_g[hs].reshape(EL, D)
        wqk = np.concatenate([wq, wk], axis=0).T           # [1024, 512]
        wvT = wv.T                                         # [1024, 256]
        # w_out columns for local features, transposed -> [256 d_loc, 1024 e]
        wo = w_out[:, 64 * 4 * g:64 * 4 * (g + 1)].T
        bias = np.zeros((128, D), dtype=np.float32)
        if g == 0:
            bias[:] = b_out[None, :]
        # pack d-tiles along the free axis: [1024, F] -> [128, 8*F]
        pack = lambda a: np.ascontiguousarray(
            a.reshape(8, 128, a.shape[1]).transpose(1, 0, 2).reshape(
                128, 8 * a.shape[1])).astype(bf16)
        xT = x[b].T                                        # [1024, 2048]
        xTp = np.ascontiguousarray(
            xT.reshape(8, 128, 4, 512).transpose(1, 2, 0, 3).reshape(
                128, 4 * 8 * 512)).astype(bf16)
        wop = np.ascontiguousarray(
            wo.reshape(2, 128, D).transpose(1, 0, 2).reshape(
                128, 2 * D)).astype(bf16)
        onesv = np.zeros((128, HL * 65), dtype=np.float32)
        onesv[:, 64::65] = 1.0
        in_maps.append({
            "ones64p": np.ones((1, 64), dtype=np.float32),
            "onesv": onesv.astype(bf16),
            "xTp": xTp,
            "wqkp": pack(wqk),
            "wvp": pack(wvT),
            "cos2": cos2,
            "sin2": sin2,
            "psw": psw,
            "id128": id128,
            "wop": wop,
            "bias": bias,
        })

    global _last_in_maps
    _last_in_maps = in_maps
    if _NC_CACHE is None:
        _NC_CACHE = build_kernel()
    res = run_spmd_per_device(_NC_CACHE, in_maps)
    outs = [res[c]["out"] for c in range(N_CORES)]
    full = np.empty((B, S, D), dtype=np.float32)
    full[0] = outs[0] + outs[1] + outs[2] + outs[3]
    full[1] = outs[4] + outs[5] + outs[6] + outs[7]
    return full


# revision 40
# speedup vs baseline: 1.1085x; 1.1085x over previous
"""Multi-head attention with Llama RoPE on 8 TRN2 NeuronCores.

Problem: x [2, 2048, 1024] f32; w_qkv [3072, 1024]; w_out [1024, 1024];
b_out [1024].  16 heads x head_dim 64, full (non-causal) softmax attention.

Sharding: 8 cores = 2 batches x 4 head-groups (4 heads per core).
Each core computes q/k/v projections for its 4 heads, RoPE, attention,
and a partial output projection over its 256 local features.  The host
sums the 4 partials per batch (row-parallel out-projection); the bias is
injected on one core per batch group.

v3 structure: engine-balanced, filler-scheduled pipeline.
 - Act engine does ONLY exp (128 x [128,1024] tiles).  psum->sbuf copies
   run on gpsimd (Pool); softmax-denominator broadcast via gpsimd
   partition_broadcast (no PE ones-matmul, no Act copy).
 - Attention: qi-chunk (1024) outer, kj-tile inner, PV lagging scores by
   one tile; Act saturates while PE runs ahead on scores.
 - v is projected TRANSPOSED (wv stationary, few LDWEIGHTS) during the
   input-DMA window, then flipped to natural layout with cheap PE
   transpose ops emitted as attention fillers.
 - All remaining projection work (q/k pair 1 + rope, out-projection
   halves T1/T2) is emitted as fillers inside the attention stream; T2
   for the first s-half runs during h3 so the tail is short.
 - x / weights / rope tables in bf16 (half DMA, same PE rate).
"""
import sys

sys.path.insert(0, "/opt/trn_rl_repo")

from collections import deque
from contextlib import ExitStack

import numpy as np

import concourse.bass as bass
import concourse.tile as tile
from concourse import bacc, mybir
from concourse.bass2jax import (_bass_exec_p, install_neuronx_cc_hook,
                                partition_id_tensor)

F32 = mybir.dt.float32
F32R = mybir.dt.float32r
BF16 = mybir.dt.bfloat16

B, S, D = 2, 2048, 1024
H, HD = 16, 64          # global heads, head dim
HL = 4                  # heads per core
EL = HL * HD            # 256 local e-dims for q, k, v each
N_CORES = 8
SCP = 512               # projection / rope / vT s-chunk
N_CP = S // SCP         # 4
SCA = 1024              # attention qi chunk
N_CA = S // SCA         # 2
N_DT = D // 128         # 8 d-chunks
N_ST = S // 128         # 16 s-tiles
N_KT = S // 128         # 16 kj-tiles


def r(ap):
    return ap.bitcast(F32R)


def build_kernel(repeat=1):
    nc = bacc.Bacc(None, target_bir_lowering=False)

    # host-packed DRAM params: partition-dim 128, d-tiles packed along the
    # free axis so each load is ONE descriptor-cheap DMA
    xT_ext = nc.declare_dram_parameter("xTp", [128, N_CP * N_DT * SCP], BF16,
                                       isOutput=False)   # [c][dt][512]
    wqk_ext = nc.declare_dram_parameter("wqkp", [128, N_DT * 2 * EL], BF16,
                                        isOutput=False)  # [dt][512]
    wv_ext = nc.declare_dram_parameter("wvp", [128, N_DT * EL], BF16,
                                       isOutput=False)   # [dt][256]
    cos2_ext = nc.declare_dram_parameter("cos2", [128, S], BF16, isOutput=False)
    sin2_ext = nc.declare_dram_parameter("sin2", [128, S], BF16, isOutput=False)
    psw_ext = nc.declare_dram_parameter("psw", [128, 128], F32R, isOutput=False)
    id128_ext = nc.declare_dram_parameter("id128", [128, 128], BF16,
                                          isOutput=False)
    wo_ext = nc.declare_dram_parameter("wop", [128, 2 * D], BF16,
                                       isOutput=False)   # [eh][1024]
    bias_ext = nc.declare_dram_parameter("bias", [128, D], F32, isOutput=False)
    ones64_ext = nc.declare_dram_parameter("ones64p", [1, 64], F32R,
                                           isOutput=False)
    onesv_ext = nc.declare_dram_parameter("onesv", [128, HL * 65], BF16,
                                          isOutput=False)
    out_ext = nc.declare_dram_parameter("out", [S, D], F32, isOutput=True)

    inv_sqrt_hd = 1.0 / np.sqrt(HD)

    with tile.TileContext(nc) as tc, ExitStack() as ctx, \
            nc.allow_low_precision(reason="bf16/f32r rounding writes"):
        # ---- persistent SBUF ----
        singles = ctx.enter_context(tc.tile_pool(name="singles", bufs=1))
        xTp = singles.tile([128, N_CP * N_DT * SCP], BF16, name="xTp")
        # xT[dt] views: free dims [chunk, 512] (stride N_DT*SCP between
        # chunks); matmul/rope operands slice these to chunk granularity
        xT4 = xTp[:].rearrange("p (c d s) -> p d c s", c=N_CP, d=N_DT)
        wqkp = singles.tile([128, N_DT * 2 * EL], BF16, name="wqkp")
        wqk = [wqkp[:, 2 * EL * i:2 * EL * (i + 1)] for i in range(N_DT)]
        wvp = singles.tile([128, N_DT * EL], BF16, name="wvp")
        wv = [wvp[:, EL * i:EL * (i + 1)] for i in range(N_DT)]
        cos2 = singles.tile([128, S], BF16, name="cos2")
        sin2 = singles.tile([128, S], BF16, name="sin2")
        psw = singles.tile([128, 128], F32R, name="psw")
        id128 = singles.tile([128, 128], BF16, name="id128")
        wop = singles.tile([128, 2 * D], BF16, name="wop")
        wo = [wop[:, D * i:D * (i + 1)] for i in range(2)]
        bias = singles.tile([128, D], F32, name="bias")
        # q/k transposed [e_local, s]; index = head pair
        qT = [singles.tile([128, S], F32, name=f"qT{i}") for i in range(2)]
        kT = [singles.tile([128, S], F32, name=f"kT{i}") for i in range(2)]
        # v transposed [e_local, s] (intermediate) and natural per s-tile
        vT = [singles.tile([128, S], BF16, name=f"vT{i}") for i in range(2)]
        vsb = [singles.tile([128, HL * 65], BF16, name=f"v{i}")
               for i in range(N_ST)]
        # normalized attention output, transposed [d_local, s]
        onrm = [singles.tile([128, S], BF16, name=f"onrm{i}")
                for i in range(2)]
        # T1 = pair-0 half of the output projection (+bias), per s-tile
        t1sb = [singles.tile([128, D], BF16, name=f"t1sb{i}")
                for i in range(N_ST)]
        ones64 = singles.tile([1, 64], F32R, name="ones64")
        warm = singles.tile([1, 2], F32, name="warm")

        # ---- DMA loads: few big DMAs (DGE descriptor gen is ~650ns each),
        # ordered so the preamble's dependencies land first
        nc.sync.dma_start(out=wvp[:], in_=wv_ext[:])
        nc.sync.dma_start(out=wqkp[:], in_=wqk_ext[:])
        nc.sync.dma_start(out=psw[:], in_=psw_ext[:])
        nc.sync.dma_start(out=id128[:], in_=id128_ext[:])
        CW = N_DT * SCP
        for c in range(N_CP):
            nc.sync.dma_start(out=xTp[:, CW * c:CW * (c + 1)],
                              in_=xT_ext[:, CW * c:CW * (c + 1)])
            if c == 1:
                nc.sync.dma_start(out=cos2[:, 0:SCA], in_=cos2_ext[:, 0:SCA])
                nc.sync.dma_start(out=sin2[:, 0:SCA], in_=sin2_ext[:, 0:SCA])
        nc.sync.dma_start(out=cos2[:, SCA:S], in_=cos2_ext[:, SCA:S])
        nc.sync.dma_start(out=sin2[:, SCA:S], in_=sin2_ext[:, SCA:S])
        nc.sync.dma_start(out=wop[:], in_=wo_ext[:])
        nc.sync.dma_start(out=bias[:], in_=bias_ext[:])
        nc.sync.dma_start(out=ones64[:], in_=ones64_ext[:])

        # ---- pools ----
        # PSUM budget (8 banks): ss 2x[128,1024] = 4, po 1x[65,1024] = 2,
        # misc 2x[128,512] = 2.
        ss_pool = ctx.enter_context(tc.tile_pool(name="ss", bufs=2, space="PSUM"))
        po_pool = ctx.enter_context(tc.tile_pool(name="po", bufs=1, space="PSUM"))
        mi_pool = ctx.enter_context(tc.tile_pool(name="mi", bufs=2, space="PSUM"))
        att_sb = ctx.enter_context(tc.tile_pool(name="att_sb", bufs=10))
        rope_sb = ctx.enter_context(tc.tile_pool(name="rope_sb", bufs=2))
        nrm_sb = ctx.enter_context(tc.tile_pool(name="nrm_sb", bufs=2))
        ob_sb = ctx.enter_context(tc.tile_pool(name="ob_sb", bufs=3))

        # ones columns of v tiles (static for the whole exec)
        for st in range(N_ST):
            nc.sync.dma_start(out=vsb[st][:], in_=onesv_ext[:])
        # warm up the Act Exp table off the critical path
        nc.vector.memset(warm[:], 0.0)
        nc.scalar.activation(out=warm[:], in_=warm[:],
                             func=mybir.ActivationFunctionType.Exp)

        # ---------- emission helpers ----------
        # generator helpers yield every ~0.5-1us of PE work so filler
        # stepping never opens a long gap in the exp stream
        def emit_vt_chunk(c):
            """vT[e 256, s-chunk c] with wv stationary (2 e-halves)."""
            sl = slice(SCP * c, SCP * (c + 1))
            for eh in range(2):
                mi = mi_pool.tile([128, SCP], F32, name="vtp", tag="mi")
                for dt in range(N_DT):
                    nc.tensor.matmul(
                        mi[:],
                        wv[dt][:, 128 * eh:128 * (eh + 1)],
                        xT4[:, dt, c, :],
                        start=(dt == 0), stop=(dt == N_DT - 1),
                    )
                    if dt % 4 == 3:
                        yield
                nc.scalar.copy(out=vT[eh][:, sl], in_=mi[:])

        def emit_vtrans(st):
            """vsb[st] natural [s, h*65] from vT via PE transpose."""
            ssl = slice(128 * st, 128 * (st + 1))
            mi = mi_pool.tile([128, SCP], F32, name="vtr", tag="mi")
            tp = mi[:, 0:128].bitcast(BF16)             # [128, 256] bf16 view
            for eh in range(2):
                nc.tensor.matmul(tp[:, 128 * eh:128 * (eh + 1)],
                                 vT[eh][:, ssl], id128[:],
                                 start=True, stop=True, is_transpose=True)
            nc.vector.tensor_copy(
                out=vsb[st][:].rearrange("p (h e) -> p h e", h=HL)[:, :, 0:64],
                in_=tp.rearrange("p (h e) -> p h e", h=HL))

        def gen_qk_tile(t):
            """Project + rope q/k tile t, one 512-col s-chunk at a time.
            Uses short-lived mi-pool psum so it can interleave with the
            attention stream without touching the scores pool."""
            buf = qT[t] if t < 2 else kT[t - 2]
            for c in range(N_CP):
                sl = slice(SCP * c, SCP * (c + 1))
                mi = mi_pool.tile([128, SCP], F32, name="qkproj", tag="mi")
                for dt in range(N_DT):
                    nc.tensor.matmul(
                        mi[:],
                        wqk[dt][:, 128 * t:128 * (t + 1)],
                        xT4[:, dt, c, :],
                        start=(dt == 0), stop=(dt == N_DT - 1),
                    )
                    if dt % 4 == 3:
                        yield
                nc.scalar.copy(out=r(buf[:, sl]), in_=mi[:])
                emit_rope_chunk(t, c)
                yield

        def emit_rope_chunk(t, c):
            """Apply rope in place on q/k buf for s-chunk c (512 cols)."""
            buf = qT[t] if t < 2 else kT[t - 2]
            sl = slice(SCP * c, SCP * (c + 1))
            sw = mi_pool.tile([128, SCP], F32, name="sw", tag="mi")
            nc.tensor.matmul(sw[:], psw[:], r(buf[:, sl]), start=True, stop=True)
            tr = rope_sb.tile([128, SCP], F32, name="ropet1")
            nc.vector.tensor_mul(tr[:], buf[:, sl], cos2[:, sl])
            nc.vector.tensor_mul(r(buf[:, sl]), sw[:], sin2[:, sl])
            nc.vector.tensor_add(r(buf[:, sl]), buf[:, sl], tr[:])

        def emit_t1(st):
            """Pair-0 half of out-projection for s-tile st, +bias, to sbuf."""
            ssl = slice(128 * st, 128 * (st + 1))
            for ec in range(2):
                esl = slice(512 * ec, 512 * (ec + 1))
                ps = mi_pool.tile([128, 512], F32, name="t1ps", tag="mi")
                nc.tensor.matmul(ps[:], onrm[0][:, ssl], wo[0][:, esl],
                                 start=True, stop=True)
                nc.vector.tensor_add(t1sb[st][:, esl], ps[:], bias[:, esl])

        def emit_t2(st):
            """Pair-1 half + T1 + store."""
            ssl = slice(128 * st, 128 * (st + 1))
            ob = ob_sb.tile([128, D], F32, name="ob")
            for ec in range(2):
                esl = slice(512 * ec, 512 * (ec + 1))
                ps = mi_pool.tile([128, 512], F32, name="t2ps", tag="mi")
                nc.tensor.matmul(ps[:], onrm[1][:, ssl], wo[1][:, esl],
                                 start=True, stop=True)
                nc.vector.tensor_add(ob[:, esl], ps[:], t1sb[st][:, esl])
            nc.sync.dma_start(out=out_ext[ssl, :], in_=ob[:])

        def run(gen):
            for _ in gen:
                pass

        # ---------- preamble ----------
        # PE p-state warmup: harmless transposes while DMAs stream; the
        # Tensor engine reaches full clock after ~3us of continuous work.
        for _ in range(24):
            mi = mi_pool.tile([128, SCP], F32, name="pwarm", tag="mi")
            nc.tensor.matmul(mi[:, 0:64].bitcast(BF16), id128[:], id128[:],
                             start=True, stop=True, is_transpose=True)
        # Preamble: pair-0 k and q projected + roped chunk-by-chunk,
        # interleaved so each chunk starts as soon as its xT DMA lands.
        gk, gq = gen_qk_tile(2), gen_qk_tile(0)
        for c in range(N_CP):
            for _ in range(3):
                next(gk, None)
            for _ in range(3):
                next(gq, None)
        run(gk)
        run(gq)

        # ---------- fillers ----------
        vsb_ready = [False] * N_ST        # vtrans emitted (PE program order)

        def gen_v():
            for c in range(N_CP):
                yield from emit_vt_chunk(c)
                for st in (4 * c, 4 * c + 2):
                    emit_vtrans(st)
                    emit_vtrans(st + 1)
                    vsb_ready[st] = vsb_ready[st + 1] = True
                    yield

        def gen_t1():
            for st in range(N_ST):
                emit_t1(st)
                yield

        def gen_t2(sts):
            for st in sts:
                emit_t2(st)
                yield

        def step(fillers, n=1):
            for _ in range(n):
                while fillers:
                    try:
                        next(fillers[0])
                        break
                    except StopIteration:
                        fillers.pop(0)

        def flush(fillers):
            while fillers:
                try:
                    next(fillers[0])
                except StopIteration:
                    fillers.pop(0)

        def emit_pv(h, po, kt, at):
            for half in range(2):
                qsl = slice(512 * half, 512 * (half + 1))
                nc.tensor.matmul(
                    po[:, qsl],
                    vsb[kt][:, 65 * h:65 * h + 65],
                    at[:, qsl],
                    start=(kt == 0), stop=(kt == N_KT - 1),
                )

        def gen_norm(h, c, po, t2_tail=False):
            """Deferred softmax normalization for a finished (h,c) block;
            stepped as the leading filler of the NEXT block so the Act
            queue keeps streaming exps while the recip/bcast chain runs."""
            tq, ro = h // 2, 64 * (h % 2)
            rec = nrm_sb.tile([1, SCA], F32, name="recip", tag="rec")
            nc.vector.reciprocal(out=r(rec[:]), in_=po[64:65, :])
            yield
            for half in range(2):
                hsl = slice(512 * half, 512 * (half + 1))
                osl = slice(SCA * c + 512 * half, SCA * c + 512 * (half + 1))
                bc = mi_pool.tile([128, 512], F32, name="bc", tag="mi")
                nc.tensor.matmul(bc[0:64, :], ones64[:], r(rec[:, hsl]),
                                 start=True, stop=True)
                bs = nrm_sb.tile([64, 512], F32, name="bcast", tag="bs")
                nc.scalar.copy(out=bs[:], in_=bc[0:64, :])
                nc.vector.tensor_mul(
                    onrm[tq][ro:ro + 64, osl], po[0:64, hsl], bs[:])
                yield
                if t2_tail:
                    for st in range(8 + 4 * half, 12 + 4 * half):
                        emit_t2(st)

        def attention_hc(h, c, fillers, steps=2, t2_tail=False):
            tq, ro = h // 2, 64 * (h % 2)
            asl = slice(SCA * c, SCA * (c + 1))
            po = po_pool.tile([65, SCA], F32, name="pv", tag="po")
            pend = deque()
            for kt in range(N_KT):
                ksl = slice(128 * kt, 128 * (kt + 1))
                ss = ss_pool.tile([128, SCA], F32, name="scores", tag="ss")
                for half in range(2):
                    qsl = slice(SCA * c + 512 * half,
                                SCA * c + 512 * (half + 1))
                    nc.tensor.matmul(
                        ss[:, 512 * half:512 * (half + 1)],
                        r(kT[tq][ro:ro + 64, ksl]),
                        r(qT[tq][ro:ro + 64, qsl]),
                        start=True, stop=True,
                    )
                at = att_sb.tile([128, SCA], BF16, name="attnT")
                nc.scalar.activation(out=at[:], in_=ss[:],
                                     func=mybir.ActivationFunctionType.Exp,
                                     scale=inv_sqrt_hd)
                pend.append((kt, at))
                step(fillers, steps)
                # PV may only be emitted once its vsb tile's producer is in
                # the PE stream (in-order engine: emitting earlier deadlocks).
                # Larger lag at the start of each (h,c): PE must not reach
                # pv(0) before the previous norm chain has released po.
                min_lag = 6 if kt < 6 else 1
                while pend and len(pend) > min_lag and vsb_ready[pend[0][0]]:
                    emit_pv(h, po, *pend.popleft())
            while pend:
                if not vsb_ready[pend[0][0]]:
                    step(fillers, 1)
                    continue
                emit_pv(h, po, *pend.popleft())
            return gen_norm(h, c, po, t2_tail=t2_tail)

        # ---------- attention schedule ----------
        fillers = [gen_v(), gen_qk_tile(3), gen_qk_tile(1)]
        norm = attention_hc(0, 0, fillers, steps=1)
        fillers.insert(0, norm)
        norm = attention_hc(0, 1, fillers, steps=1)
        fillers.insert(0, norm)
        norm = attention_hc(1, 0, fillers, steps=1)
        fillers.insert(0, norm)
        norm = attention_hc(1, 1, fillers, steps=1)
        flush(fillers)
        run(norm)                         # h1-c1 norm before T1 fillers
        fillers = [gen_t1()]
        norm = attention_hc(2, 0, fillers, steps=1)
        fillers.insert(0, norm)
        norm = attention_hc(2, 1, fillers, steps=1)
        fillers.insert(0, norm)
        norm = attention_hc(3, 0, fillers, steps=1)
        flush(fillers)                    # T1 complete before T2 starts
        run(norm)                         # h3-c0 norm before its T2 fillers
        fillers = [gen_t2(range(8))]
        norm = attention_hc(3, 1, fillers, steps=1, t2_tail=True)
        flush(fillers)
        run(norm)

    nc.finalize()
    return nc


def run_spmd_per_device(nc, in_maps):
    """8 independent single-device executions of the same NEFF (the kernel
    is pure SPMD, no collectives; the axon terminal here hangs on
    multi-device shard_map, so we dispatch per-device jits asynchronously
    instead)."""
    import jax
    install_neuronx_cc_hook()
    devs = jax.devices()[:len(in_maps)]
    partition_name = (nc.partition_id_tensor.name
                      if nc.partition_id_tensor else None)
    in_names, out_names, out_avals, zero_outs = [], [], [], []
    for alloc in nc.m.functions[0].allocations:
        if not isinstance(alloc, mybir.MemoryLocationSet):
            continue
        name = alloc.memorylocations[0].name
        if alloc.kind == "ExternalInput":
            if name != partition_name:
                in_names.append(name)
        elif alloc.kind == "ExternalOutput":
            shape = tuple(alloc.tensor_shape)
            dtype = mybir.dt.np(alloc.dtype)
            out_names.append(name)
            out_avals.append(jax.core.ShapedArray(shape, dtype))
            zero_outs.append(np.zeros(shape, dtype))
    n_params = len(in_names)
    all_names = in_names + out_names
    if partition_name is not None:
        all_names = all_names + [partition_name]

    def _body(*args):
        operands = list(args)
        if partition_name is not None:
            operands.append(partition_id_tensor())
        outs = _bass_exec_p.bind(
            *operands,
            out_avals=tuple(out_avals),
            in_names=tuple(all_names),
            out_names=tuple(out_names),
            lowering_input_output_aliases=(),
            sim_require_finite=True,
            sim_require_nnan=True,
            nc=nc,
        )
        return tuple(outs)

    donate = tuple(range(n_params, n_params + len(out_names)))
    pending = []
    for i, in_map in enumerate(in_maps):
        f = jax.jit(_body, donate_argnums=donate, keep_unused=True,
                    device=devs[i])
        args = [np.asarray(in_map[k]) for k in in_names]
        args += [z.copy() for z in zero_outs]
        pending.append(f(*args))
    return [{name: np.asarray(outs[i]) for i, name in enumerate(out_names)}
            for outs in pending]


_ROPE_TABLES = None


def _tables():
    global _ROPE_TABLES
    if _ROPE_TABLES is None:
        bf16 = mybir.dt.np(BF16)
        inv_freq = 1.0 / (10000.0 ** (np.arange(0, HD, 2, dtype=np.float32) / HD))
        t = np.arange(S, dtype=np.float32)
        freqs = np.outer(t, inv_freq).astype(np.float32)  # [S, 32]
        cos, sin = np.cos(freqs), np.sin(freqs)
        # interleave pairs: row 2i and 2i+1 both get cos_i; sin row 2i = -s_i,
        # row 2i+1 = +s_i; tile 2 heads to fill 128 partitions
        c64 = np.repeat(cos.T, 2, axis=0)                 # [64, S]
        s64 = np.repeat(sin.T, 2, axis=0).copy()
        s64[0::2, :] *= -1.0
        cos2 = np.tile(c64, (2, 1)).astype(bf16)          # [128, S]
        sin2 = np.tile(s64, (2, 1)).astype(bf16)
        psw = np.zeros((128, 128), dtype=np.float32)
        idx = np.arange(128)
        psw[idx ^ 1, idx] = 1.0                           # out[j] = in[j^1]
        _ROPE_TABLES = (cos2, sin2, psw,
                        np.eye(128, dtype=np.float32).astype(bf16))
    return _ROPE_TABLES


_NC_CACHE = None
_last_in_maps = None


def kernel(x, w_qkv, w_out, b_out):
    global _NC_CACHE
    bf16 = mybir.dt.np(BF16)
    x = np.ascontiguousarray(x, dtype=np.float32)
    w_qkv = np.asarray(w_qkv, dtype=np.float32)
    w_out = np.asarray(w_out, dtype=np.float32)
    b_out = np.asarray(b_out, dtype=np.float32)

    cos2, sin2, psw, id128 = _tables()
    wq_g = w_qkv[0 * D:1 * D].reshape(H, HD, D)
    wk_g = w_qkv[1 * D:2 * D].reshape(H, HD, D)
    wv_g = w_qkv[2 * D:3 * D].reshape(H, HD, D)

    in_maps = []
    for c in range(N_CORES):
        b, g = divmod(c, 4)
        hs = slice(4 * g, 4 * g + 4)
        wq = wq_g[hs].reshape(EL, D)                       # [256, 1024]
        wk = wk_g[hs].reshape(EL, D)
        wv = wv_g[hs].reshape(EL, D)
        wqk = np.concatenate([wq, wk], axis=0).T           # [1024, 512]
        wvT = wv.T                                         # [1024, 256]
        # w_out columns for local features, transposed -> [256 d_loc, 1024 e]
        wo = w_out[:, 64 * 4 * g:64 * 4 * (g + 1)].T
        bias = np.zeros((128, D), dtype=np.float32)
        if g == 0:
            bias[:] = b_out[None, :]
        # pack d-tiles along the free axis: [1024, F] -> [128, 8*F]
        pack = lambda a: np.ascontiguousarray(
            a.reshape(8, 128, a.shape[1]).transpose(1, 0, 2).reshape(
                128, 8 * a.shape[1])).astype(bf16)
        xT = x[b].T                                        # [1024, 2048]
        xTp = np.ascontiguousarray(
            xT.reshape(8, 128, 4, 512).transpose(1, 2, 0, 3).reshape(
                128, 4 * 8 * 512)).astype(bf16)
        wop = np.ascontiguousarray(
            wo.reshape(2, 128, D).transpose(1, 0, 2).reshape(
                128, 2 * D)).astype(bf16)
        onesv = np.zeros((128, HL * 65), dtype=np.float32)
        onesv[:, 64::65] = 1.0
        in_maps.append({
            "ones64p": np.ones((1, 64), dtype=np.float32),
            "onesv": onesv.astype(bf16),
            "xTp": xTp,
            "wqkp": pack(wqk),
            "wvp": pack(wvT),
            "cos2": cos2,
            "sin2": sin2,
            "psw": psw,
            "id128": id128,
            "wop": wop,
            "bias": bias,
        })

    global _last_in_maps
    _last_in_maps = in_maps
    if _NC_CACHE is None:
        _NC_CACHE = build_kernel()
    res = run_spmd_per_device(_NC_CACHE, in_maps)
    outs = [res[c]["out"] for c in range(N_CORES)]
    full = np.empty((B, S, D), dtype=np.float32)
    full[0] = outs[0] + outs[1] + outs[2] + outs[3]
    full[1] = outs[4] + outs[5] + outs[6] + outs[7]
    return full
